# revision 1
# baseline (speedup 1.0000x reference)
"""Trainium2 Bass kernel for nn_DynamicFilter (dynamic per-image 3x3 grouped filter).

Math (per batch n, channel c, group g = c//4):
    pooled[n,c] = mean_hw x[n,c]
    f = pooled @ W2.T + b2          (conv1x1 + folded BN)
    filt[n,g,k] = tanh(f)           (k = 9 taps, 3x3, reflect pad)
    out = A_c * conv3x3_{filt[g]}(x) + s_c * x + Bc_c * pooled[n,c]
      A = lamb_l*(inside_all+1), s = lamb_h+1, Bc = -lamb_l*inside_all

Sharding: 8 cores = (n in 0..3) x (channel half in 0..1), 16 channels/core.
The pooled mean needs all 32 channels per n -> tiny pair AllGather (16 floats).

Device mapping per core:
  - x rows (H) on SBUF partitions, one window-tile set per channel with
    1-row overlaps and reflection rows/cols materialized at load.
  - 3x3 conv = 3 matmuls per channel accumulating in PSUM: each lhsT is a
    tridiagonal [in_row x out_row] matrix carrying the 3 vertical taps for
    one horizontal shift dx; rhs free-dim offset provides dx.
  - residual s*x folded into the center tridiagonal diagonal as sigma=s/A;
    per-channel scale A and bias Bc*pooled applied by the ScalarE copy that
    evacuates PSUM.
  - pooling: per-window ones-vector matmuls -> column sums in PSUM ->
    free-dim reduce -> [16,1] partial -> pair AllGather -> [32,1].
"""

import numpy as np

import concourse.bass as bass
import concourse.mybir as mybir
import concourse.tile as tile
from concourse import bacc, bass_utils

F32 = mybir.dt.float32
F32R = mybir.dt.float32r

N_B, C, H, W = 4, 32, 256, 256
CPC = 16   # channels per core
NCORES = 8
EPS = 1e-5

# smallrow / broadcast-table column layout
FCOL = 0     # filt (own 4 groups x 9 taps) : 36
ACOL = 36    # A_eff per own channel        : 16
SIGCOL = 52  # sigma = s/A_eff              : 16
BCCOL = 68   # Bc = -lamb_l*inside_all      : 16
SCOL = 84    # s = lamb_h+1 (unused on dev) : 16
POOLCOL = 100  # own pooled sums            : 16
SRW = 116

# window geometry: (main_src_row0, main_nrows, main_dst_part0,
#                   reflect_src_row, reflect_dst_part,
#                   pool_part0, pool_part1, out_row0, out_nrows, in_parts)
WINDOWS = [
    (0, 87, 1, 1, 0, 1, 87, 0, 86, 88),     # rows -1..86 (part0 = reflect row 1)
    (85, 87, 0, None, None, 1, 86, 86, 85, 87),
    (170, 86, 0, 254, 86, 1, 86, 171, 85, 87),  # part86 = reflect row 254
]
WBLK = 264  # column stride of one window block inside a channel tile
# pooling selector block inside the shifts constant: M[p, 384 + 15 + 16w] is
# the 0/1 row-valid mask of window w; slicing 16 cols starting at
# 384 + 15 + 16w - ch puts that mask at relative column ch, zeros elsewhere.
SELBASE = 384
SHIFTS_W = 128


def _build_nc():
    nc = bacc.Bacc(num_swdge_queues=4)
    xw = nc.declare_dram_parameter("xw", [128, 3 * CPC * WBLK], F32R, isOutput=False)
    w2t = nc.declare_dram_parameter("w2t", [C, 36], F32, isOutput=False)
    params = nc.declare_dram_parameter("params", [1, 100], F32, isOutput=False)
    shifts = nc.declare_dram_parameter("shifts", [128, SHIFTS_W], F32, isOutput=False)
    selmask = nc.declare_dram_parameter("selmask", [128, 64], F32R, isOutput=False)
    out_d = nc.declare_dram_parameter("out", [86, 3 * CPC * W], F32, isOutput=True)

    cc_in = nc.dram_tensor("cc_in", [1, CPC], F32)
    cc_out = nc.dram_tensor("cc_out", [1, C], F32)

    CQ = 4           # channels per quad
    NQ = CPC // CQ   # quads
    OBLK = 3 * W     # out-tile columns per channel (w0|w1|w2)

    with tile.TileContext(nc) as tc:
        with (
            tc.tile_pool(name="wbig", bufs=1) as wpool,
            tc.tile_pool(name="consts", bufs=1) as cpool,
            tc.tile_pool(name="small", bufs=1) as spool,
            tc.tile_pool(name="tri", bufs=1) as tripool,
            tc.tile_pool(name="tritmp", bufs=2) as tmppool,
            tc.tile_pool(name="outs", bufs=1) as opool,
            tc.tile_pool(name="ps_small", bufs=1, space="PSUM") as pspool,
            tc.tile_pool(name="ps_a", bufs=4, space="PSUM") as psa_pool,
            tc.tile_pool(name="ps_b", bufs=2, space="PSUM") as psb_pool,
        ):
            # constants
            shifts_t = cpool.tile([128, SHIFTS_W], F32, tag="shifts")
            nc.sync.dma_start(shifts_t[:, :], shifts[:, :])
            w2t_t = cpool.tile([C, 36], F32, tag="w2t")
            nc.sync.dma_start(w2t_t[:, :], w2t[:, :])
            selmask_t = cpool.tile([128, 64], F32R, tag="selmask")
            nc.sync.dma_start(selmask_t[:, :], selmask[:, :])
            ptile = cpool.tile([1, 100], F32, tag="ptile")
            nc.sync.dma_start(ptile[:, :], params[:, :])
            onesrow = cpool.tile([1, 128], F32, tag="onesrow")
            nc.vector.memset(onesrow[:, :], 1.0)
            smallrow = cpool.tile([1, SRW], F32, tag="smallrow")
            nc.sync.dma_start(smallrow[0:1, ACOL:SCOL + 16], ptile[0:1, 36:100])

            # window tensor (w, c, x): host pre-builds the full window
            # layout (row windows, overlaps, reflect rows+cols) so it loads as
            # 3 flat contiguous DMAs at line rate.
            wcvt = wpool.tile([128, 3 * CPC * WBLK], F32R, tag="wcvt")
            t4 = wcvt[:, :].rearrange("p (w c x) -> p w c x", c=CPC, x=WBLK)

            def on_q(inst, qn):
                if qn:
                    inst.ins.queue = f"qPoolDynamic{qn}"
                return inst

            WB = CPC * WBLK
            HB = WB // 2
            HC = CPC // 2
            colsums = spool.tile([128, 48], F32R, tag="colsums")
            for wi in range(3):
                nparts = WINDOWS[wi][9]
                for h in range(2):
                    on_q(nc.gpsimd.dma_start(
                        wcvt[0:128, wi * WB + h * HB:wi * WB + (h + 1) * HB],
                        xw[:, wi * WB + h * HB:wi * WB + (h + 1) * HB],
                    ), (2 * wi + h) % 4)
                    # per-(row,channel) column sums on DVE (PE stays free)
                    with nc.allow_low_precision(reason="fp32r column sums"):
                        nc.vector.tensor_reduce(
                            colsums[0:nparts, wi * CPC + h * HC:wi * CPC + (h + 1) * HC],
                            t4[0:nparts, wi, h * HC:(h + 1) * HC, 1:257].bitcast(F32),
                            axis=mybir.AxisListType.X, op=mybir.AluOpType.add,
                        )
            # masked cross-row sums -> pooled row [1, 16] in PSUM
            prow_ps = pspool.tile([1, CPC], F32, tag="ps3")
            for wi in range(3):
                mcol = 15 + 16 * wi
                nc.tensor.matmul(
                    prow_ps[:, :],
                    selmask_t[0:128, mcol:mcol + 1],
                    colsums[0:128, wi * CPC:(wi + 1) * CPC],
                    start=(wi == 0),
                    stop=(wi == 2),
                )

            # pooled row -> smallrow + AllGather
            nc.scalar.activation(
                smallrow[0:1, POOLCOL:POOLCOL + 16], prow_ps[:, :],
                mybir.ActivationFunctionType.Copy,
            )
            nc.sync.dma_start(cc_in[:, :], smallrow[0:1, POOLCOL:POOLCOL + 16])
            nc.gpsimd.collective_compute(
                "AllGather",
                mybir.AluOpType.bypass,
                replica_groups=[[0, 1], [2, 3], [4, 5], [6, 7]],
                ins=[cc_in.ap().opt()],
                outs=[cc_out.ap().opt()],
            )
            pooled_col = spool.tile([C, 1], F32, tag="pooled_col")
            nc.sync.dma_start(pooled_col[:, :], cc_out[:, :])

            # f = pooled @ W2s.T ; filt = tanh(f + b2)
            f_ps = pspool.tile([1, 36], F32, tag="ps3")
            nc.tensor.matmul(f_ps[:, :], pooled_col[:, :], w2t_t[:, :])
            fb = spool.tile([1, 36], F32, tag="fb")
            nc.vector.tensor_add(fb[:, :], f_ps[:, :], ptile[0:1, 0:36])
            nc.scalar.activation(
                smallrow[0:1, 0:36], fb[:, :], mybir.ActivationFunctionType.Tanh
            )

            # broadcast table: every smallrow value replicated down 128 partitions
            bct_ps = pspool.tile([128, SRW], F32, tag="ps3")
            nc.tensor.matmul(bct_ps[:, :], onesrow[:, :], smallrow[:, :])
            bct = spool.tile([128, SRW], F32, tag="bct")
            nc.scalar.activation(bct[:, :], bct_ps[:, :], mybir.ActivationFunctionType.Copy)
            bcol = spool.tile([128, CPC], F32, tag="bcol")
            nc.vector.tensor_mul(
                bcol[:, :], bct[:, BCCOL:BCCOL + 16], bct[:, POOLCOL:POOLCOL + 16]
            )

            # main conv + evac per channel; tridiag builds interleaved so
            # the DVE FIFO isn't clogged ahead of the evacuations.
            ot = opool.tile([86, 3 * CPC * W], F32, tag="ot")
            ot4 = ot[:, :].rearrange("p (c w x) -> p c w x", w=3, x=W)
            traw = {}
            tc0 = {}
            for ch in range(CPC):
                g = ch // 4
                if ch % 4 == 0:
                    for dxi, dx in enumerate((-1, 0, 1)):
                        wm = FCOL + 9 * g + (dx + 1)
                        w0c = wm + 3
                        wp = wm + 6
                        t1 = tmppool.tile([88, 86], F32, tag="t1")
                        nc.vector.tensor_scalar_mul(
                            t1[:, :], shifts_t[0:88, 0:86], bct[0:88, wm:wm + 1]
                        )
                        t2 = tmppool.tile([88, 86], F32, tag="t2")
                        nc.vector.scalar_tensor_tensor(
                            t2[:, :], shifts_t[0:88, 1:87], bct[0:88, w0c:w0c + 1],
                            t1[:, :], op0=mybir.AluOpType.mult, op1=mybir.AluOpType.add,
                        )
                        tr = tripool.tile([88, 86], F32R, tag=f"traw{g}_{dxi}")
                        nc.vector.scalar_tensor_tensor(
                            tr[:, :], shifts_t[0:88, 2:88], bct[0:88, wp:wp + 1],
                            t2[:, :], op0=mybir.AluOpType.mult, op1=mybir.AluOpType.add,
                        )
                        traw[(g, dxi)] = tr
                t = tripool.tile([88, 86], F32R, tag=f"tc0_{ch}")
                nc.vector.scalar_tensor_tensor(
                    t[:, :], shifts_t[0:88, 1:87], bct[0:88, SIGCOL + ch:SIGCOL + ch + 1],
                    traw[(g, 1)][:, :], op0=mybir.AluOpType.mult, op1=mybir.AluOpType.add,
                )
                tc0[ch] = t
                psa = psa_pool.tile([86, 512], F32, tag="psa")
                if ch % 2 == 0:
                    psb2 = psb_pool.tile([85, 512], F32, tag="psb")
                    psb2_saved = psb2
                else:
                    psb2 = psb2_saved
                half = (ch % 2) * W
                for dxi, dx in enumerate((-1, 0, 1)):
                    lt = tc0[ch] if dx == 0 else traw[(g, dxi)]
                    nc.tensor.matmul(
                        psa[:, :],
                        lt[0:88, 0:86],
                        t4[0:88, 0:2, ch, dx + 1:dx + 257],
                        start=(dxi == 0),
                        stop=(dxi == 2),
                    )
                    if dx == 0:
                        nc.tensor.matmul(
                            psb2[0:85, half:half + W],
                            lt[0:87, 0:85],
                            t4[0:87, 2, ch, dx + 1:dx + 257],
                            start=False,
                            stop=(ch % 2 == 1),
                        )
                    elif ch % 2 == 0:
                        # paired: w2 of ch and ch+1 share the group lhsT
                        nc.tensor.matmul(
                            psb2[0:85, :],
                            lt[0:87, 0:85],
                            t4[0:87, 2, ch:ch + 2, dx + 1:dx + 257],
                            start=(dxi == 0),
                            stop=False,
                        )
                nc.vector.tensor_scalar(
                    ot4[0:86, ch, 0:2, :],
                    psa[:, :].rearrange("p (a b) -> p a b", b=W),
                    bct[0:86, ACOL + ch:ACOL + ch + 1],
                    bcol[0:86, ch:ch + 1],
                    op0=mybir.AluOpType.mult,
                    op1=mybir.AluOpType.add,
                )
                if ch % 2 == 1:
                    for c2 in (ch - 1, ch):
                        h2 = (c2 % 2) * W
                        nc.scalar.activation(
                            ot4[0:85, c2, 2, :], psb2[0:85, h2:h2 + W],
                            mybir.ActivationFunctionType.Identity,
                            bias=bcol[0:85, c2:c2 + 1],
                            scale=bct[0:85, ACOL + c2:ACOL + c2 + 1],
                        )
                if ch % 4 == 3:
                    q0 = ch - 3
                    c0 = q0 * 3 * W
                    on_q(nc.gpsimd.dma_start(
                        out_d[0:86, c0:c0 + 12 * W],
                        ot[0:86, c0:c0 + 12 * W],
                    ), (q0 // 4) % 4)

    nc.compile()
    return nc


_NC_CACHE = None


def _get_nc():
    global _NC_CACHE
    if _NC_CACHE is None:
        _NC_CACHE = _build_nc()
    return _NC_CACHE


def _selmask_np():
    s = np.zeros((128, 64), np.float32)
    for wi, (_, _, _, _, _, pv0, pv1, _, _, _) in enumerate(WINDOWS):
        s[pv0:pv1, 15 + 16 * wi] = 1.0
    return s


def _shifts_np():
    s = np.zeros((128, SHIFTS_W), np.float32)
    for p in range(128):
        s[p, p] = 1.0  # identity block; I_d = cols d:d+86 of rows 0:88
    return s


# row indices per window (length 128; tail rows unused -> clamp to 0)
def _win_rows():
    rows = []
    for wi, (r0, nr, p0, rr, rp, _, _, _, _, nparts) in enumerate(WINDOWS):
        idx = np.zeros(128, np.int64)
        idx[p0:p0 + nr] = np.arange(r0, r0 + nr)
        if rr is not None:
            idx[rp] = rr
        rows.append((idx, nparts))
    return rows


_WIN_ROWS = _win_rows()


def _build_windows(xs_np):
    """xs_np [16, 256, 256] fp32 -> [128, 3*16*264] window layout."""
    out = np.zeros((128, 3, CPC, WBLK), np.float32)
    for wi, (idx, nparts) in enumerate(_WIN_ROWS):
        g = xs_np[:, idx[:nparts], :]          # [16, nparts, 256]
        g = np.ascontiguousarray(g.transpose(1, 0, 2))  # [nparts, 16, 256]
        out[:nparts, wi, :, 1:257] = g
        out[:nparts, wi, :, 0] = g[:, :, 1]
        out[:nparts, wi, :, 257] = g[:, :, 254]
    return out.reshape(128, 3 * CPC * WBLK)


def _scatter_out(flat, dst):
    """flat [86, 16*3*256] (c, w, x) -> dst [16, 256, 256]."""
    f = flat.reshape(86, CPC, 3, W)
    dst[:, 0:86, :] = f[0:86, :, 0].transpose(1, 0, 2)
    dst[:, 86:171, :] = f[0:85, :, 1].transpose(1, 0, 2)
    dst[:, 171:256, :] = f[0:85, :, 2].transpose(1, 0, 2)


def kernel(x, conv_w, bn_gamma, bn_beta, bn_mean, bn_var, lamb_l, lamb_h, inside_all):
    x = np.asarray(x, np.float32)
    conv_w = np.asarray(conv_w, np.float32)
    bn_gamma = np.asarray(bn_gamma, np.float32)
    bn_beta = np.asarray(bn_beta, np.float32)
    bn_mean = np.asarray(bn_mean, np.float32)
    bn_var = np.asarray(bn_var, np.float32)
    lamb_l = np.asarray(lamb_l, np.float32)
    lamb_h = np.asarray(lamb_h, np.float32)
    ia = np.asarray(inside_all, np.float32).reshape(C)

    gv = (bn_gamma / np.sqrt(bn_var + np.float32(EPS))).astype(np.float32)
    w2s = (conv_w * gv[:, None] / np.float32(H * W)).astype(np.float32)  # [72, 32]
    b2 = (bn_beta - bn_mean * gv).astype(np.float32)                      # [72]

    A = (lamb_l * (ia + 1.0)).astype(np.float32)
    s = (lamb_h + 1.0).astype(np.float32)
    # device bias multiplies Bc by the pooled SUM, so fold the mean's 1/HW here
    Bc = (-lamb_l * ia / np.float32(H * W)).astype(np.float32)
    A_eff = np.where(A >= 0, np.maximum(A, 1e-20), np.minimum(A, -1e-20)).astype(np.float32)
    sig = (s / A_eff).astype(np.float32)

    shifts = _shifts_np()
    selmask = _selmask_np()
    nc = _get_nc()

    in_maps = []
    for core in range(NCORES):
        n = core // 2
        half = core % 2
        csl = slice(16 * half, 16 * half + 16)
        gsl = slice(36 * half, 36 * half + 36)
        params = np.concatenate(
            [b2[gsl], A_eff[csl], sig[csl], Bc[csl], s[csl]]
        ).astype(np.float32).reshape(1, 100)
        in_maps.append({
            "xw": _build_windows(x[n, csl]),
            "w2t": np.ascontiguousarray(w2s[gsl].T),
            "params": params,
            "shifts": shifts,
            "selmask": selmask,
        })

    res = bass_utils.run_bass_kernel_spmd(nc, in_maps, core_ids=list(range(NCORES)))

    out = np.empty((N_B, C, H, W), np.float32)
    for core in range(NCORES):
        n = core // 2
        half = core % 2
        _scatter_out(res.results[core]["out"], out[n, 16 * half:16 * half + 16])
    return out

